# revision 1
# baseline (speedup 1.0000x reference)
"""Trainium2 Bass kernel for a 5-layer MLP over graph nodes (ChebConv K=1 == Linear).

Network: x[50000,512] -> ELU(x@W1+b1) -> ... -> h@W5+b5, dims 512->2048(x4)->256.
The ChebConv(K=1) branch and parallel Linear branch fuse on the host into a
single weight W = Wg + Wl, bias b = bg + bl.  edge_index is unused (no
neighbor exchange for K=1).

Sharding: data-parallel over nodes, 6250 nodes per core on 8 NeuronCores,
weights replicated.  No collectives.

Per-core dataflow (all static, fully unrolled, Tile framework):
  - activations live feature-major in SBUF: H^T tiles [128 feat, Kc, NB nodes]
  - input x is pre-transposed/pre-cast on the host to feature-major bf16
    [128, 4, NPC] and DMA'd straight into SBUF (no device-side transpose)
  - layers 1-4: out^T[m] = sum_k W[k,m]^T @ H^T[k]   (lhsT = weight block,
    moving = activations, PSUM fp32 accumulate), ELU fused on eviction:
    elu(z) = relu(z) + exp(min(z,0)) - 1   (ACT: relu,exp; DVE: min,add,add)
  - layer 5 flips the mapping: lhsT = H^T chunk (stationary), moving = W5
    -> PSUM comes out node-major [128 nodes, 256], copied and DMA'd out.
  - weights are streamed from DRAM per 1024-node block (activations never
    leave SBUF between layers).
"""

import numpy as np
import ml_dtypes

N = 50000
IN_C = 512
DIM = 2048
OUT_C = 256
NCORES = 8
NPC = N // NCORES  # 6250 nodes per core
NB = 1024  # node block size
BLOCKS = [NB] * (NPC // NB) + ([NPC % NB] if NPC % NB else [])  # [1024]*6 + [106]
LAYER_DIMS = [(IN_C, DIM), (DIM, DIM), (DIM, DIM), (DIM, DIM), (DIM, OUT_C)]

_cache = {}


def _build():
    import concourse.bass as bass
    import concourse.tile as tile
    from concourse import bacc, mybir
    f32 = mybir.dt.float32
    bf16 = mybir.dt.bfloat16
    AF = mybir.ActivationFunctionType
    ALU = mybir.AluOpType

    nc = bacc.Bacc(
        trn_type="TRN2", target_bir_lowering=False, debug=False, num_devices=NCORES
    )

    x_h = nc.dram_tensor("x", [128, IN_C // 128, NPC], bf16, kind="ExternalInput")
    # mid-layer weights, host-packed as [16 mblk, 128 part, Kc, 128] bf16
    w_h = []
    for l, (ci, co) in enumerate(LAYER_DIMS[:4], start=1):
        kc = ci // 128
        w_h.append(
            nc.dram_tensor(f"w{l}", [co // 128, 128, kc, 128], bf16, kind="ExternalInput")
        )
    # layer-5 weight, host-packed as [128 part, 16 kblk, 256] bf16
    w5_h = nc.dram_tensor("w5", [128, DIM // 128, OUT_C], bf16, kind="ExternalInput")
    b_h = [
        nc.dram_tensor(f"b{l}", [128, DIM // 128], f32, kind="ExternalInput")
        for l in range(1, 5)
    ]
    out_h = nc.dram_tensor("out", [NPC, OUT_C], f32, kind="ExternalOutput")

    x_ap = x_h.ap()
    out_ap = out_h.ap()

    with tile.TileContext(nc) as tc:
        from contextlib import ExitStack

        with ExitStack() as ctx:
            const = ctx.enter_context(tc.tile_pool(name="const", bufs=1))
            a0p = ctx.enter_context(tc.tile_pool(name="a0", bufs=2))
            actp = ctx.enter_context(tc.tile_pool(name="acts", bufs=2))
            wp = ctx.enter_context(tc.tile_pool(name="wp", bufs=5))
            etp = ctx.enter_context(tc.tile_pool(name="et", bufs=3))
            outp = ctx.enter_context(tc.tile_pool(name="outp", bufs=3))
            psp = ctx.enter_context(tc.tile_pool(name="ps", bufs=8, space="PSUM"))

            w5sb = const.tile([128, DIM // 128, OUT_C], bf16)
            nc.sync.dma_start(w5sb[:], w5_h.ap())
            bsb = []
            for l in range(4):
                bt = const.tile([128, DIM // 128], f32, tag=f"b{l}")
                nc.sync.dma_start(bt[:], b_h[l].ap())
                bsb.append(bt)

            n0 = 0
            for nb in BLOCKS:
                nch = (nb + 127) // 128  # 128-node chunks in this block

                # ---- input stage: host pre-transposed/pre-cast x, direct DMA
                h0 = a0p.tile([128, IN_C // 128, NB], bf16, tag="h0")
                nc.sync.dma_start(h0[:, :, :nb], x_ap[:, :, n0 : n0 + nb])

                # ---- layers 1..4 (feature-major, ELU)
                hin = h0
                for li in range(4):
                    kc = LAYER_DIMS[li][0] // 128
                    hout = actp.tile([128, DIM // 128, NB], bf16, tag="hout")
                    for m in range(DIM // 128):
                        wt = wp.tile([128, 16, 128], bf16, tag="wt")
                        nc.sync.dma_start(wt[:, :kc, :], w_h[li].ap()[m])
                        bias = bsb[li][:, m : m + 1]
                        for ch in range((nb + 511) // 512):
                            cs = min(512, nb - ch * 512)
                            sl = slice(ch * 512, ch * 512 + cs)
                            pz = psp.tile([128, 512], f32, tag="pz")
                            for k in range(kc):
                                nc.tensor.matmul(
                                    pz[:, :cs],
                                    wt[:, k, :],
                                    hin[:, k, sl],
                                    start=(k == 0),
                                    stop=(k == kc - 1),
                                )
                            # ELU eviction: relu(z+b) + exp(min(z+b,0)) - 1
                            r = etp.tile([128, 512], f32, tag="r")
                            nc.scalar.activation(
                                r[:, :cs], pz[:, :cs], AF.Relu, bias=bias, scale=1.0
                            )
                            mn = etp.tile([128, 512], f32, tag="mn")
                            nc.vector.tensor_scalar(
                                mn[:, :cs], pz[:, :cs], bias, 0.0, ALU.add, ALU.min
                            )
                            ex = etp.tile([128, 512], f32, tag="ex")
                            nc.scalar.activation(ex[:, :cs], mn[:, :cs], AF.Exp)
                            tt = etp.tile([128, 512], f32, tag="tt")
                            nc.vector.tensor_add(tt[:, :cs], r[:, :cs], ex[:, :cs])
                            nc.vector.tensor_scalar(
                                hout[:, m, sl], tt[:, :cs], -1.0, None, ALU.add
                            )
                    hin = hout

                # ---- layer 5: node-major output, no activation
                for c in range(nch):
                    csz = min(128, nb - c * 128)
                    c0 = c * 128
                    p5 = psp.tile([128, 512], f32, tag="pz")
                    for k in range(DIM // 128):
                        nc.tensor.matmul(
                            p5[:csz, :OUT_C],
                            hin[:, k, c0 : c0 + csz],
                            w5sb[:, k, :],
                            start=(k == 0),
                            stop=(k == DIM // 128 - 1),
                        )
                    ot = outp.tile([128, OUT_C], f32, tag="ot")
                    nc.scalar.copy(ot[:csz, :], p5[:csz, :OUT_C])
                    nc.sync.dma_start(
                        out_ap[n0 + c0 : n0 + c0 + csz, :], ot[:csz, :]
                    )

                n0 += nb

    nc.compile()
    return nc


def _prep_weights(inputs):
    bf16 = ml_dtypes.bfloat16
    wmaps = {}
    for l, (ci, co) in enumerate(LAYER_DIMS, start=1):
        W = np.asarray(inputs[f"Wg{l}"], np.float32) + np.asarray(
            inputs[f"Wl{l}"], np.float32
        )
        b = np.asarray(inputs[f"bg{l}"], np.float32) + np.asarray(
            inputs[f"bl{l}"], np.float32
        )
        if l < 5:
            kc = ci // 128
            wt = np.ascontiguousarray(
                W.reshape(kc, 128, co // 128, 128).transpose(2, 1, 0, 3)
            ).astype(bf16)
            wmaps[f"w{l}"] = wt
            wmaps[f"b{l}"] = np.ascontiguousarray(b.reshape(co // 128, 128).T)
        else:
            wmaps["w5"] = np.ascontiguousarray(
                W.reshape(ci // 128, 128, co).transpose(1, 0, 2)
            ).astype(bf16)
            wmaps["_b5"] = b
    return wmaps


LAST_RESULTS = None


def make_in_maps(inputs):
    x = np.asarray(inputs["x"], np.float32)
    assert x.shape == (N, IN_C)
    wmaps = _prep_weights(inputs)
    b5 = wmaps.pop("_b5")
    in_maps = []
    for c in range(NCORES):
        xs = x[c * NPC : (c + 1) * NPC]
        # [p, f, n] with value x[n, f*128+p], bf16
        xt = np.ascontiguousarray(
            xs.T.reshape(IN_C // 128, 128, NPC).transpose(1, 0, 2)
        ).astype(ml_dtypes.bfloat16)
        m = {"x": xt}
        m.update(wmaps)
        in_maps.append(m)
    return in_maps, b5


def kernel(**inputs) -> np.ndarray:
    global LAST_RESULTS

    from concourse.bass_utils import run_bass_kernel_spmd

    in_maps, b5 = make_in_maps(inputs)

    if "nc" not in _cache:
        _cache["nc"] = _build()
    nc = _cache["nc"]

    res = run_bass_kernel_spmd(nc, in_maps, core_ids=list(range(NCORES)))
    LAST_RESULTS = res
    out = np.concatenate([res.results[c]["out"] for c in range(NCORES)], axis=0)
    if np.any(b5):
        out = out + b5[None, :]
    return np.ascontiguousarray(out.astype(np.float32))



# revision 8
# speedup vs baseline: 1.5824x; 1.5824x over previous
"""Trainium2 raw-Bass kernel for a 5-layer MLP over graph nodes (ChebConv K=1).

Network: x[50000,512] -> ELU(x@W1+b1) -> ... -> h@W5+b5, dims 512->2048(x4)->256.
ChebConv(K=1) + parallel Linear fuse on the host: W = Wg+Wl, b = bg+bl.
edge_index is unused (no neighbor exchange for K=1).

Sharding: data-parallel over nodes, 6250 nodes/core on 8 NeuronCores, weights
replicated, no collectives.

Implementation notes (raw Bass, manual semaphores — no Tile framework):
  - activations live feature-major in SBUF as [128 feat-part, kblk, nodes] bf16;
    x is host-pre-transposed/cast so no device transpose is needed.
  - node blocks of 1536/1642; within a block, mid layers run
    m(16) x ch(nodes/N_MM) x k(kc) matmul groups into 6 rotating PSUM slots,
    one slot per bank (bank-exclusive => no PE-W/DVE-R same-bank hazards).
  - ELU eviction: r=relu(z+b) [ACT], mn=min(z+b,0) [DVE], e=exp(mn) [ACT],
    h = (r-1)+e [DVE scalar_tensor_tensor] -> bf16, 2 ops/engine per tile.
  - layer 5 flips the mapping (lhsT = activation chunk, moving = W5) to give
    node-major [<=128 nodes, 256] PSUM tiles, copied by ACT and DMA'd out.
  - weights stream from DRAM through 6 rotating SBUF buffers, re-fetched per
    node block (4 blocks => 4x26MB, ~12% DMA duty, fully hidden).
  - `passes` re-runs the whole computation; used by test.py to measure the
    steady-state device time as (T(passes=2) - T(passes=1)) with the (large,
    noisy) axon dispatch cost cancelled.
"""

import numpy as np
import ml_dtypes

N = 50000
IN_C = 512
DIM = 2048
OUT_C = 256
NCORES = 8
NPC = N // NCORES  # 6250
BLOCKS = [1536, 1536, 1536, 1642]
assert sum(BLOCKS) == NPC
NBMAX = max(BLOCKS)
LAYER_DIMS = [(IN_C, DIM), (DIM, DIM), (DIM, DIM), (DIM, DIM), (DIM, OUT_C)]
KCS = [4, 16, 16, 16]  # contraction 128-blocks for layers 1..4
NWBUF = 6  # rotating weight buffers
NMID_PS = 6  # PSUM slots for mid layers (one per bank 0..5)

_cache = {}


def _chunks(total, step):
    out = []
    o = 0
    while o < total:
        c = min(step, total - o)
        out.append((o, c))
        o += c
    return out


def _build(n_mm=512, passes=1):
    import concourse.bass as bass
    from concourse import bacc, mybir

    f32 = mybir.dt.float32
    bf16 = mybir.dt.bfloat16
    AF = mybir.ActivationFunctionType
    ALU = mybir.AluOpType

    nc = bacc.Bacc(
        trn_type="TRN2", target_bir_lowering=False, debug=False, num_devices=NCORES
    )

    x_h = nc.dram_tensor("x", [128, IN_C // 128, NPC], bf16, kind="ExternalInput")
    w_h = []
    for l, (ci, co) in enumerate(LAYER_DIMS[:4], start=1):
        kc = ci // 128
        w_h.append(
            nc.dram_tensor(f"w{l}", [co // 128, 128, kc, 128], bf16, kind="ExternalInput")
        )
    w5_h = nc.dram_tensor("w5", [128, DIM // 128, OUT_C], bf16, kind="ExternalInput")
    # biases for layers 1..4 as [128, 5, 16] (layer-1..4 cols + a zeros col)
    b_h = nc.dram_tensor("bmin", [128, 5, 16], f32, kind="ExternalInput")
    out_h = nc.dram_tensor("out", [NPC, OUT_C], f32, kind="ExternalOutput")

    x_ap = x_h.ap()
    out_ap = out_h.ap()

    # ---------- static schedules (shared by all engine programs) ----------
    # weight tile stream: per block, (layer_idx 0..3, m 0..15); identical
    # every block (weights are re-fetched per block)
    WT_PER_BLOCK = [(li, m) for li in range(4) for m in range(16)]

    # mid eviction tiles in PE order: per block, layers 1..4, m, ch
    def mid_tiles_of_block(b):
        nb = BLOCKS[b]
        tiles = []
        for li in range(4):
            for m in range(16):
                for ch, (off, cs) in enumerate(_chunks(nb, n_mm)):
                    tiles.append((b, li, m, off, cs))
        return tiles

    MID_TILES = [mid_tiles_of_block(b) for b in range(len(BLOCKS))]
    L5_CHUNKS = [_chunks(BLOCKS[b], 128) for b in range(len(BLOCKS))]
    NBLK = len(BLOCKS)
    n_wt_pass = NBLK * len(WT_PER_BLOCK)
    n_mid_pass = sum(len(t) for t in MID_TILES)
    n_l5_pass = sum(len(c) for c in L5_CHUNKS)
    block_n0 = np.cumsum([0] + BLOCKS).tolist()

    # cumulative mid-group count after each weight tile / after each L1,
    # used by the DMA program to recycle buffers off the single s_pm counter
    wt_release = []   # per global wt index: s_pm value when its m-tile is done
    x_release = []    # per global block g: s_pm value when L1 of g is done
    _mc = 0
    for _p in range(passes):
        for _b in range(NBLK):
            _ch = len(_chunks(BLOCKS[_b], n_mm))
            for _li in range(4):
                for _m in range(16):
                    _mc += _ch
                    wt_release.append(_mc)
                if _li == 0:
                    x_release.append(_mc)

    from contextlib import ExitStack

    with ExitStack() as ctx:
        xf = ctx.enter_context(nc.sbuf_tensor("xf", [128, 2, IN_C // 128, NBMAX], bf16))
        hb = ctx.enter_context(nc.sbuf_tensor("hb", [128, 2, DIM // 128, NBMAX], bf16))
        wb = ctx.enter_context(nc.sbuf_tensor("wb", [128, NWBUF, 16, 128], bf16))
        w5sb = ctx.enter_context(nc.sbuf_tensor("w5sb", [128, DIM // 128, OUT_C], bf16))
        bsb = ctx.enter_context(nc.sbuf_tensor("bsb", [128, 5, 16], f32))
        rt = ctx.enter_context(nc.sbuf_tensor("rt", [128, 2, 512], f32))
        mnt = ctx.enter_context(nc.sbuf_tensor("mnt", [128, 2, 512], f32))
        ext = ctx.enter_context(nc.sbuf_tensor("ext", [128, 2, 512], f32))
        osb = ctx.enter_context(nc.sbuf_tensor("osb", [128, 4, OUT_C], f32))
        pz = ctx.enter_context(nc.psum_tensor("pz", [128, NMID_PS, 512], f32))
        p5 = ctx.enter_context(nc.psum_tensor("p5", [128, 2, 512], f32))
        s_cw = ctx.enter_context(nc.semaphore("s_cw"))  # const DMAs done (inc 16)
        s_xs = [ctx.enter_context(nc.semaphore(f"s_x{i}")) for i in range(2)]
        s_wb = [ctx.enter_context(nc.semaphore(f"s_wb{i}")) for i in range(NWBUF)]
        s_pm = ctx.enter_context(nc.semaphore("s_pm"))  # PE mid psum group complete
        s_r = ctx.enter_context(nc.semaphore("s_r"))    # ACT relu done
        s_mn = ctx.enter_context(nc.semaphore("s_mn"))  # DVE min done
        s_ex = ctx.enter_context(nc.semaphore("s_ex"))  # ACT exp done
        s_ev = ctx.enter_context(nc.semaphore("s_ev"))  # DVE stt (eviction) done
        s_p5 = ctx.enter_context(nc.semaphore("s_p5"))  # PE L5 psum chunk complete
        s_oc = ctx.enter_context(nc.semaphore("s_oc"))  # ACT out-copy done
        s_ods = [ctx.enter_context(nc.semaphore(f"s_od{i}")) for i in range(4)]
        block = ctx.enter_context(nc.Block())
        zero_ap = bsb[:, 4, 0:1]

        @block.sync
        def _(sync):
            # consts
            sync.dma_start(bsb[:], b_h.ap()).then_inc(s_cw, 16)
            sync.dma_start(w5sb[:], w5_h.ap()).then_inc(s_cw, 16)
            wt = 0          # weight dma index (global)
            oj = 0          # out chunk dma index (global)
            for p in range(passes):
                for b in range(NBLK):
                    g = p * NBLK + b
                    n0 = block_n0[b]
                    # x for block g (slot g%2); slot last read by L1 of g-2
                    if g >= 2:
                        sync.wait_ge(s_pm, x_release[g - 2])
                    sync.dma_start(
                        xf[:, g % 2, :, : BLOCKS[b]], x_ap[:, :, n0 : n0 + BLOCKS[b]]
                    ).then_inc(s_xs[g % 2], 16)
                    # first NWBUF weight tiles of block g (their back-pressure
                    # waits resolve during block g-1, so no head-of-line block)
                    for li, m in WT_PER_BLOCK[:NWBUF]:
                        if wt >= NWBUF:
                            sync.wait_ge(s_pm, wt_release[wt - NWBUF])
                        kc = KCS[li]
                        sync.dma_start(
                            wb[:, wt % NWBUF, :kc, :], w_h[li].ap()[m]
                        ).then_inc(s_wb[wt % NWBUF], 16)
                        wt += 1
                    # out DMAs of previous block (b==0's predecessor is
                    # handled by the previous pass's tail section)
                    if b >= 1:
                        bp = b - 1
                        n0p = block_n0[bp]
                        for c0, csz in L5_CHUNKS[bp]:
                            sync.wait_ge(s_oc, oj + 1)
                            sync.dma_start(
                                out_ap[n0p + c0 : n0p + c0 + csz, :],
                                osb[:csz, oj % 4, :],
                            ).then_inc(s_ods[oj % 4], 16)
                            oj += 1
                    # remaining weight tiles for block g
                    for li, m in WT_PER_BLOCK[NWBUF:]:
                        sync.wait_ge(s_pm, wt_release[wt - NWBUF])
                        kc = KCS[li]
                        sync.dma_start(
                            wb[:, wt % NWBUF, :kc, :], w_h[li].ap()[m]
                        ).then_inc(s_wb[wt % NWBUF], 16)
                        wt += 1
                # final block's out DMAs
                bp = NBLK - 1
                n0p = block_n0[bp]
                for c0, csz in L5_CHUNKS[bp]:
                    sync.wait_ge(s_oc, oj + 1)
                    sync.dma_start(
                        out_ap[n0p + c0 : n0p + c0 + csz, :], osb[:csz, oj % 4, :]
                    ).then_inc(s_ods[oj % 4], 16)
                    oj += 1
            for i in range(4):
                cnt = oj // 4 + (1 if oj % 4 > i else 0)
                if cnt:
                    sync.wait_ge(s_ods[i], 16 * cnt)

        @block.tensor
        def _(tensor):
            tensor.wait_ge(s_cw, 32)
            wt = 0
            mt = 0
            jc = 0
            for p in range(passes):
                for b in range(NBLK):
                    g = p * NBLK + b
                    nb = BLOCKS[b]
                    tensor.wait_ge(s_xs[g % 2], 16 * (g // 2 + 1))
                    layer_base = mt
                    for li in range(4):
                        kc = KCS[li]
                        if li > 0:
                            # all evictions of previous layer must be done
                            tensor.wait_ge(s_ev, layer_base)
                        layer_base = mt + 16 * len(_chunks(nb, n_mm))
                        hi = (li - 1) % 2  # input h slot for li>=1
                        ho = li % 2
                        for m in range(16):
                            tensor.wait_ge(s_wb[wt % NWBUF], 16 * (wt // NWBUF + 1))
                            last = None
                            for off, cs in _chunks(nb, n_mm):
                                if mt >= NMID_PS:
                                    tensor.wait_ge(s_ev, mt - (NMID_PS - 1))
                                slot = mt % NMID_PS
                                for k in range(kc):
                                    src = (
                                        xf[:, g % 2, k, off : off + cs]
                                        if li == 0
                                        else hb[:, hi, k, off : off + cs]
                                    )
                                    last = tensor.matmul(
                                        pz[:, slot, :cs],
                                        wb[:, wt % NWBUF, k, :],
                                        src,
                                        start=(k == 0),
                                        stop=(k == kc - 1),
                                    )
                                last.then_inc(s_pm, 1)
                                mt += 1
                            wt += 1
                    # L5: wait for all mid evictions of this block
                    tensor.wait_ge(s_ev, mt)
                    for c0, csz in L5_CHUNKS[b]:
                        if jc >= 2:
                            tensor.wait_ge(s_oc, jc - 1)
                        last = None
                        for k in range(DIM // 128):
                            last = tensor.matmul(
                                p5[:csz, jc % 2, :OUT_C],
                                hb[:, 1, k, c0 : c0 + csz],
                                w5sb[:, k, :],
                                start=(k == 0),
                                stop=(k == DIM // 128 - 1),
                            )
                        last.then_inc(s_p5, 1)
                        jc += 1

        @block.scalar
        def _(scalar):
            scalar.wait_ge(s_cw, 32)
            mt = 0
            jc = 0
            for p in range(passes):
                for b in range(NBLK):
                    nb = BLOCKS[b]
                    for li in range(4):
                        bias_col = li
                        for m in range(16):
                            bias = bsb[:, bias_col, m : m + 1]
                            for off, cs in _chunks(nb, n_mm):
                                slot = mt % NMID_PS
                                if mt >= 2:
                                    scalar.wait_ge(s_ev, mt - 1)
                                scalar.wait_ge(s_pm, mt + 1)
                                scalar.activation(
                                    rt[:, mt % 2, :cs],
                                    pz[:, slot, :cs],
                                    AF.Relu,
                                    bias=bias,
                                    scale=1.0,
                                ).then_inc(s_r, 1)
                                scalar.wait_ge(s_mn, mt + 1)
                                scalar.activation(
                                    ext[:, mt % 2, :cs],
                                    mnt[:, mt % 2, :cs],
                                    AF.Exp,
                                    bias=zero_ap,
                                    scale=1.0,
                                ).then_inc(s_ex, 1)
                                mt += 1
                    # L5 psum -> sbuf copies
                    for c0, csz in L5_CHUNKS[b]:
                        scalar.wait_ge(s_p5, jc + 1)
                        if jc >= 4:
                            scalar.wait_ge(s_ods[jc % 4], 16 * ((jc - 4) // 4 + 1))
                        scalar.copy(
                            osb[:csz, jc % 4, :], p5[:csz, jc % 2, :OUT_C]
                        ).then_inc(s_oc, 1)
                        jc += 1

        @block.vector
        def _(vector):
            vector.wait_ge(s_cw, 32)
            mt = 0
            prev = None  # (slot, cs, hout AP) of tile mt-1
            for p in range(passes):
                for b in range(NBLK):
                    nb = BLOCKS[b]
                    for li in range(4):
                        ho = li % 2
                        for m in range(16):
                            bias = bsb[:, li, m : m + 1]
                            for off, cs in _chunks(nb, n_mm):
                                if prev is not None:
                                    pslot, pcs, pout, pmt = prev
                                    vector.wait_ge(s_ex, pmt + 1)
                                    vector.scalar_tensor_tensor(
                                        pout,
                                        rt[:, pmt % 2, :pcs],
                                        -1.0,
                                        ext[:, pmt % 2, :pcs],
                                        ALU.add,
                                        ALU.add,
                                    ).then_inc(s_ev, 1)
                                vector.wait_ge(s_r, mt + 1)
                                vector.tensor_scalar(
                                    mnt[:, mt % 2, :cs],
                                    pz[:, mt % NMID_PS, :cs],
                                    bias,
                                    0.0,
                                    ALU.add,
                                    ALU.min,
                                ).then_inc(s_mn, 1)
                                hout = hb[:, ho, m, off : off + cs]
                                prev = (mt % NMID_PS, cs, hout, mt)
                                mt += 1
            # flush final eviction
            pslot, pcs, pout, pmt = prev
            vector.wait_ge(s_ex, pmt + 1)
            vector.scalar_tensor_tensor(
                pout,
                rt[:, pmt % 2, :pcs],
                -1.0,
                ext[:, pmt % 2, :pcs],
                ALU.add,
                ALU.add,
            ).then_inc(s_ev, 1)

    nc.compile()
    return nc


def _prep_weights(inputs):
    bf16 = ml_dtypes.bfloat16
    wmaps = {}
    bmin = np.zeros((128, 5, 16), np.float32)
    for l, (ci, co) in enumerate(LAYER_DIMS, start=1):
        W = np.asarray(inputs[f"Wg{l}"], np.float32) + np.asarray(
            inputs[f"Wl{l}"], np.float32
        )
        b = np.asarray(inputs[f"bg{l}"], np.float32) + np.asarray(
            inputs[f"bl{l}"], np.float32
        )
        if l < 5:
            kc = ci // 128
            wt = np.ascontiguousarray(
                W.reshape(kc, 128, co // 128, 128).transpose(2, 1, 0, 3)
            ).astype(bf16)
            wmaps[f"w{l}"] = wt
            bmin[:, l - 1, :] = b.reshape(co // 128, 128).T
        else:
            wmaps["w5"] = np.ascontiguousarray(
                W.reshape(ci // 128, 128, co).transpose(1, 0, 2)
            ).astype(bf16)
            wmaps["_b5"] = b
    wmaps["bmin"] = bmin
    return wmaps


def make_in_maps(inputs):
    x = np.asarray(inputs["x"], np.float32)
    assert x.shape == (N, IN_C)
    wmaps = _prep_weights(inputs)
    b5 = wmaps.pop("_b5")
    in_maps = []
    for c in range(NCORES):
        xs = x[c * NPC : (c + 1) * NPC]
        # [p, f, n] with value x[n, f*128+p], bf16
        xt = np.ascontiguousarray(
            xs.T.reshape(IN_C // 128, 128, NPC).transpose(1, 0, 2)
        ).astype(ml_dtypes.bfloat16)
        m = {"x": xt}
        m.update(wmaps)
        in_maps.append(m)
    return in_maps, b5


def kernel(**inputs) -> np.ndarray:
    from concourse.bass_utils import run_bass_kernel_spmd

    in_maps, b5 = make_in_maps(inputs)

    if "nc" not in _cache:
        _cache["nc"] = _build()
    nc = _cache["nc"]

    res = run_bass_kernel_spmd(nc, in_maps, core_ids=list(range(NCORES)))
    out = np.concatenate([res.results[c]["out"] for c in range(NCORES)], axis=0)
    if np.any(b5):
        out = out + b5[None, :]
    return np.ascontiguousarray(out.astype(np.float32))
